# revision 37
# baseline (speedup 1.0000x reference)
"""Trainium2 Bass kernel for the AdaptPrompt segment-reduce problem.

Computation (see reference):
    counts/centers/delta = per-class segment means over 10000 few-shot rows
    xr = Q1_x[remaining_idxes]                       # [190000, 256] gather
    sim = softmax(normalize(xr) @ normalize(centers).T)
    out = xr + sim @ delta

Strategy: host dedups the remaining-row support (~61% of table rows are
referenced) and value-range shards the unique rows across 8 cores; the
device streams its rows with NO gather; host applies out[i]=dev[rem[i]].

Device pipeline per 512-row group, 9-deep software pipelined (every
cross-engine handoff lands >=1 iteration before its consumer), d-major
end to end so the PSUM drains are wide and scalar-free:
  P0 [PE]   logits q[c, r] = cn8^T @ x8 (fp8 DoubleRow, stationary cn8T
            constant -> trivial weight loads); x8 is host-normalized so
            q is already the cosine logits.
  P1 [ACT]  e = exp(q) straight off PSUM -> fp8 SBUF.
  P2 [PE]   den_rep[16, r] = ones16x16^T @ e: the denominator lands
            REPLICATED on 16 partitions in one matmul.
  P3 [ACT]  den -> SBUF f32 (reciprocal_approx_fast is SBUF-only).
  P4 [DVE]  rden = reciprocal_approx_fast(den) (~18 bits, 5x cheaper
            than exact; hw TT divide does not exist).
  P5 [Pool] eh = e * rden on the otherwise-idle GpSimd engine --
            normalizing the [16, r] WEIGHTS is 16x cheaper than scaling
            the [128, r] output (and needs no per-row scalars).
  P6 [PE]   apply co[d-half, r] = (8*delta_h)^T @ eh, fp8 stationary.
  P7 [DVE]  ob = co*(1/8) + xt in ONE wide scalar_tensor_tensor per
            group; SP DMAs ob out per 1024-row pair.
The few-shot segment reduction is replicated on every core (an
AllReduce costs a ~41us all-core barrier on this fabric): one fp8
DoubleRow one-hot matmul per 256-row pair with rhs [x1 | x2-x1];
counts come from a one-hot row-sum matmul; class stats (centers,
cosine-normalized cn8T fp8, 8x delta fp8) are computed once.
"""

import os
from contextlib import ExitStack

import numpy as np
import ml_dtypes

import concourse.bass as bass
import concourse.mybir as mybir
import concourse.tile as tile
from concourse.bacc import Bacc

DT = mybir.dt
ALU = mybir.AluOpType
ACTF = mybir.ActivationFunctionType
BF = DT.bfloat16
FP8 = DT.float8e4

CORES = 8
N, D, NUM = 200000, 256, 16
S = 10000
S_PAIRS = 40                # few-shot 256-row tile-pairs (10240 padded rows)
FS_W = 512                  # [x1(256) | x2-x1(256)]
FS_CH = 10                  # few-shot tile-pairs per DMA chunk
GRAN = 512                  # main-loop rows per pipeline iteration


def build_nc(rp):
    h1_dve = os.environ.get("KDBG_H1", "dve") == "dve"    # drain engine mode
    eh_pool = os.environ.get("KDBG_EH", "pool") == "pool"  # eh-mult engine
    dp_apply = os.environ.get("KDBG_DP", "1") == "1"      # fp8 apply pipeline
    pe_heat = os.environ.get("KDBG_HEAT", "") == "1"      # PE warm-up filler

    nc = Bacc(target_bir_lowering=False, num_devices=CORES)

    x8d = nc.declare_dram_parameter("x8d", [128, 2, rp], FP8, isOutput=False)
    xq_t = nc.declare_dram_parameter("xq_t", [128, 2, rp], BF, isOutput=False)
    x12 = nc.declare_dram_parameter("x12", [128, S_PAIRS, 2, FS_W], FP8,
                                    isOutput=False)
    yf = nc.declare_dram_parameter("yf", [128, S_PAIRS, 2], DT.float32,
                                   isOutput=False)
    out = nc.declare_dram_parameter("out", [128, 2, rp], BF, isOutput=True)

    with tile.TileContext(nc) as tc, ExitStack() as ctx:
        cpool = ctx.enter_context(tc.tile_pool(name="const", bufs=1))

        # ---- few-shot DMAs first: everything downstream waits on them ----
        yf_sb = cpool.tile([128, S_PAIRS, 2], DT.float32)
        nc.sync.dma_start(out=yf_sb[:], in_=yf[:, :, :])
        fsp = ctx.enter_context(tc.tile_pool(name="fs", bufs=4))
        fs_tiles = []
        for ch in range(S_PAIRS // FS_CH):
            x_c = fsp.tile([128, FS_CH, 2, FS_W], FP8, name="x_c")
            nc.sync.dma_start(
                out=x_c[:], in_=x12[:, ch * FS_CH:(ch + 1) * FS_CH, :, :])
            fs_tiles.append(x_c)

        # ---- constants ----
        ident_f = cpool.tile([128, 128], DT.float32)
        from concourse.masks import make_identity
        make_identity(nc, ident_f[:])
        ident_bf = cpool.tile([128, 128], BF)
        nc.vector.tensor_copy(ident_bf[:], ident_f[:])
        iota_i = cpool.tile([128, NUM], DT.int32)
        nc.gpsimd.iota(iota_i[:], pattern=[[1, NUM]], base=0, channel_multiplier=0)
        iota_f = cpool.tile([128, NUM], DT.float32)
        nc.vector.tensor_copy(iota_f[:], iota_i[:])
        ones_sq = cpool.tile([NUM, NUM], FP8 )
        nc.vector.memset(ones_sq[:], 1.0)

        # resident main-loop inputs
        x8_all = ctx.enter_context(tc.tile_pool(name="x8a", bufs=1)).tile(
            [128, 2, rp], FP8, name="x8_all")
        xt_all = ctx.enter_context(tc.tile_pool(name="xta", bufs=1)).tile(
            [128, 2, rp], BF, name="xt_all")

        cn8T = cpool.tile([128, 2, NUM], FP8)
        delta_aug = cpool.tile([NUM, D], BF)
        delta8 = cpool.tile([NUM, D], FP8)

        # ---- phase 1: few-shot per-class segment sums (replicated);
        # one fp8 DoubleRow matmul per 256-row pair sums x1 AND x2-x1 ----
        with tc.tile_pool(name="fsp", bufs=1, space="PSUM") as fsps:
            cs_ps = fsps.tile([NUM, FS_W], DT.float32, name="cs_ps")
            ct_ps = fsps.tile([1, NUM], DT.float32, name="ct_ps")
            DR = mybir.MatmulPerfMode.DoubleRow
            oh_all = cpool.tile([128, S_PAIRS, 2, NUM], FP8)
            nc.vector.tensor_tensor(
                out=oh_all[:],
                in0=yf_sb[:, :, :, None].to_broadcast([128, S_PAIRS, 2, NUM]),
                in1=iota_f[:, None, None, :]
                    .to_broadcast([128, S_PAIRS, 2, NUM]),
                op=ALU.is_equal)
            for a in range(S_PAIRS):
                x_c = fs_tiles[a // FS_CH]
                st, sp = (a == 0), (a == S_PAIRS - 1)
                nc.tensor.matmul(cs_ps[:], lhsT=oh_all[:, a, :, :],
                                 rhs=x_c[:, a % FS_CH, :, :], start=st,
                                 stop=sp, perf_mode=DR)

            # ---- phase 2: class stats; counts from a one-hot row-sum ----
            oh2 = cpool.tile([128, NUM, S_PAIRS, 2], DT.float32)
            nc.vector.tensor_tensor(
                out=oh2[:],
                in0=yf_sb[:, None, :, :]
                    .to_broadcast([128, NUM, S_PAIRS, 2]),
                in1=iota_f[:, :, None, None]
                    .to_broadcast([128, NUM, S_PAIRS, 2]),
                op=ALU.is_equal)
            ohsum = cpool.tile([128, NUM], DT.float32)
            nc.vector.tensor_reduce(out=ohsum[:], in_=oh2[:],
                                    axis=mybir.AxisListType.XY, op=ALU.add)
            ones_col = cpool.tile([128, 1], BF)
            nc.vector.memset(ones_col[:], 1.0)
            ohsum_bf = cpool.tile([128, NUM], BF)
            nc.vector.tensor_copy(ohsum_bf[:], ohsum[:])
            nc.tensor.matmul(ct_ps[:], lhsT=ones_col[:], rhs=ohsum_bf[:],
                             start=True, stop=True)
            ctT = fsps.tile([NUM, 1], BF, name="ctT")
            ct_bf = cpool.tile([1, NUM], BF)
            nc.vector.tensor_copy(ct_bf[:], ct_ps[:])
            nc.tensor.transpose(ctT[:], in_=ct_bf[:],
                                identity=ident_bf[0:1, 0:1])
            rc = cpool.tile([NUM, 1], DT.float32)
            nc.vector.reciprocal(rc[:], ctT[:])
            centers = cpool.tile([NUM, D], DT.float32)
            nc.vector.tensor_scalar_mul(centers[:], cs_ps[:, 0:256], rc[:])
            nc.vector.tensor_scalar_mul(delta_aug[:], cs_ps[:, 256:512], rc[:])
            cscr = cpool.tile([NUM, D], DT.float32)
            nc.vector.tensor_tensor(
                out=cscr[:], in0=centers[:], in1=centers[:], op=ALU.mult)
            csum = cpool.tile([NUM, 1], DT.float32)
            nc.vector.tensor_reduce(
                out=csum[:], in_=cscr[:], axis=mybir.AxisListType.X, op=ALU.add)
            clog = cpool.tile([NUM, 1], DT.float32)
            nc.scalar.activation(out=clog[:], in_=csum[:], func=ACTF.Ln)
            cinv = cpool.tile([NUM, 1], DT.float32)
            nc.scalar.activation(out=cinv[:], in_=clog[:], func=ACTF.Exp,
                                 scale=-0.5)
            cn_bf = cpool.tile([NUM, D], BF)
            nc.vector.tensor_scalar_mul(cn_bf[:], centers[:], cinv[:])
            # 8x-scaled fp8 delta for the DoublePixel apply (the 1/8 is
            # folded into the drain STT scalar)
            nc.scalar.mul(delta8[:], delta_aug[:], 8.0)
            ctp = fsps.tile([128, 2, NUM], BF, name="ctp")
            for h in range(2):
                nc.tensor.transpose(ctp[:, h, :],
                                    in_=cn_bf[:, h * 128:(h + 1) * 128],
                                    identity=ident_bf[0:NUM, 0:NUM])
            nc.vector.tensor_copy(cn8T[:], ctp[:])

        # resident-table DMA: paired 2048-row super-chunks
        for g in range((rp + 2047) // 2048):
            c0 = g * 2048
            w = min(rp - c0, 2048)
            nc.sync.dma_start(out=x8_all[:, :, c0:c0 + w],
                              in_=x8d[:, :, c0:c0 + w])
            nc.sync.dma_start(out=xt_all[:, :, c0:c0 + w],
                              in_=xq_t[:, :, c0:c0 + w])

        # ---- phase 3: streaming main loop, 512-row groups ----
        # 8-stage software pipeline, deepest stage emitted first: every
        # cross-engine handoff is >=1 iteration ahead of its consumer.
        qps = ctx.enter_context(tc.tile_pool(name="qps", bufs=2, space="PSUM"))
        dps = ctx.enter_context(tc.tile_pool(name="dps", bufs=2, space="PSUM"))
        cps = ctx.enter_context(tc.tile_pool(name="cps", bufs=2, space="PSUM"))
        smp = ctx.enter_context(tc.tile_pool(name="sm", bufs=7))
        dnp = ctx.enter_context(tc.tile_pool(name="dn", bufs=3))
        rdp = ctx.enter_context(tc.tile_pool(name="rd", bufs=3))
        ehp = ctx.enter_context(tc.tile_pool(name="eh", bufs=4))
        obp = ctx.enter_context(tc.tile_pool(name="ob", bufs=2))

        ngr = rp // GRAN
        stash = {}
        ob_tiles = {}
        DRm = mybir.MatmulPerfMode.DoubleRow
        DPm = mybir.MatmulPerfMode.DoublePixel if dp_apply else None
        dl = delta8 if dp_apply else delta_aug
        ksc = 0.125 if dp_apply else 1.0

        def stage_p0(g):          # PE: logits, stationary cn8T
            q = qps.tile([NUM, GRAN], DT.float32, name="q")
            if pe_heat:
                # dependency-free warm-up matmul: keeps the PE busy enough
                # to hold its boost clock (overwritten by the real one)
                nc.tensor.matmul(
                    q[:], lhsT=cn8T[:], rhs=x8_all[:, :, 0:GRAN],
                    start=True, stop=True, perf_mode=DRm,
                    skip_group_check=True)
            nc.tensor.matmul(
                q[:], lhsT=cn8T[:],
                rhs=x8_all[:, :, g * GRAN:(g + 1) * GRAN],
                start=True, stop=True, perf_mode=DRm,
                skip_group_check=True)
            stash[g] = q

        def stage_p1(g):          # ACT: exp off PSUM
            e8 = smp.tile([NUM, GRAN], FP8 if dp_apply else BF, name="e8")
            nc.scalar.activation(out=e8[:], in_=stash.pop(g)[:],
                                 func=ACTF.Exp)
            stash[(g, "e")] = e8

        def stage_p2(g):          # PE: replicated den rows (ones^T @ e)
            dq = dps.tile([NUM, GRAN], DT.float32, name="dq")
            nc.tensor.matmul(dq[:], lhsT=ones_sq[:],
                             rhs=stash[(g, "e")][:], start=True, stop=True,
                             perf_mode=DPm)
            stash[(g, "q2")] = dq

        def stage_p3(g):          # ACT: den PSUM -> SBUF f32
            dn = dnp.tile([NUM, GRAN], DT.float32, name="dn")
            nc.scalar.copy(dn[:], stash.pop((g, "q2"))[:])
            stash[(g, "d")] = dn

        def stage_p4(g):          # DVE: fast reciprocal (SBUF only)
            rdn = rdp.tile([NUM, GRAN], DT.float32, name="rdn")
            nc.vector.reciprocal_approx_fast(rdn[:], stash.pop((g, "d"))[:])
            stash[(g, "r")] = rdn

        def stage_p5(g):          # Pool: eh = e * rden
            e8 = stash.pop((g, "e"))
            rdn = stash.pop((g, "r"))
            eh = ehp.tile([NUM, GRAN], FP8 if dp_apply else BF, name="eh")
            nc.gpsimd.tensor_tensor(out=eh[:], in0=e8[:], in1=rdn[:],
                                    op=ALU.mult)
            stash[(g, "h")] = eh

        def stage_p6(g):          # PE: apply, stationary delta halves
            eh = stash.pop((g, "h"))
            co = cps.tile([128, 2, GRAN], DT.float32, name="co")
            for h in range(2):
                nc.tensor.matmul(co[:, h, :],
                                 lhsT=dl[:, h * 128:(h + 1) * 128],
                                 rhs=eh[:], start=True, stop=True,
                                 perf_mode=DPm)
            stash[g] = co

        def stage_p7(g):          # DVE: wide fused drain; SP: DMA per pair
            co = stash.pop(g)
            if g % 2 == 0:
                ob_tiles[g // 2] = obp.tile([128, 2, 2 * GRAN], BF, name="ob")
            ob = ob_tiles[g // 2]
            m = g % 2
            nc.vector.scalar_tensor_tensor(
                out=ob[:, :, m * GRAN:(m + 1) * GRAN], in0=co[:, :, :],
                scalar=ksc, in1=xt_all[:, :, g * GRAN:(g + 1) * GRAN],
                op0=ALU.mult, op1=ALU.add)
            if m == 1 or g == ngr - 1:
                p0 = (g // 2) * 2 * GRAN
                w = min(rp - p0, 2 * GRAN)
                nc.sync.dma_start(out=out[:, :, p0:p0 + w],
                                  in_=ob_tiles.pop(g // 2)[:, :, 0:w])

        stages = [(0, stage_p0), (8, stage_p7), (7, stage_p6), (1, stage_p1),
                  (2, stage_p2), (3, stage_p3), (4, stage_p4), (5, stage_p5)]
        depth = 9
        for it in range(ngr + depth - 1):
            for off, fn in stages:   # deepest stage first
                kk = it - off
                if 0 <= kk < ngr:
                    fn(kk)
    nc.finalize()
    return nc


def _shard_inputs(Q1_x, Q2_x, Q1_y, selected_idxes, remaining_idxes):
    """Host-side glue: few-shot layout, dedup of the remaining-row support,
    value-range sharding of the unique rows across cores."""
    bf16 = ml_dtypes.bfloat16
    fp8 = ml_dtypes.float8_e4m3
    Q1_x = np.asarray(Q1_x, dtype=np.float32)
    Q2_x = np.asarray(Q2_x, dtype=np.float32)
    y = np.asarray(Q1_y).astype(np.int32)
    sel = np.asarray(selected_idxes).astype(np.int64)
    rem = np.asarray(remaining_idxes).astype(np.int64)

    uniq, inv = np.unique(rem, return_inverse=True)
    bounds = np.searchsorted(uniq, np.arange(CORES + 1) * (N // CORES))
    ncounts = np.diff(bounds)
    rp = int(max(1, -(-int(ncounts.max()) // GRAN))) * GRAN

    # few-shot block: [x1(256) | x2-x1(256)]
    s_pad = S_PAIRS * 256
    v = np.zeros((s_pad, FS_W), dtype=np.float32)
    v[:S, 0:256] = Q1_x[sel]
    v[:S, 256:512] = Q2_x[sel] - Q1_x[sel]
    x12 = np.ascontiguousarray(
        v.reshape(S_PAIRS, 2, 128, FS_W).transpose(2, 0, 1, 3).astype(fp8))
    yv = np.full((s_pad,), -1.0, dtype=np.float32)
    yv[:S] = y[sel].astype(np.float32)
    yfa = np.ascontiguousarray(yv.reshape(S_PAIRS, 2, 128).transpose(2, 0, 1))

    in_maps = []
    for c in range(CORES):
        rows_c = uniq[bounds[c]:bounds[c + 1]]
        xs = np.ones((rp, D), dtype=np.float32)
        xs[:len(rows_c)] = Q1_x[rows_c]
        rn = 1.0 / np.sqrt(np.einsum("rd,rd->r", xs, xs))
        # d-major [p, h, r] = x[r, h*128+p]
        xtf = np.ascontiguousarray(xs.T.reshape(2, 128, rp).transpose(1, 0, 2))
        in_maps.append({"x8d": (xtf * rn).astype(fp8),
                        "xq_t": xtf.astype(bf16),
                        "x12": x12, "yf": yfa})
    return in_maps, rp, bounds, inv, len(uniq)


def kernel(Q1_x, Q2_x, Q1_y, selected_idxes, remaining_idxes, num, _bench=None):
    from concourse.bass_utils import run_bass_kernel_spmd

    in_maps, rp, bounds, inv, nuniq = _shard_inputs(
        Q1_x, Q2_x, Q1_y, selected_idxes, remaining_idxes)
    nc = build_nc(rp)
    kwargs = dict(_bench or {})
    res = run_bass_kernel_spmd(nc, in_maps, core_ids=list(range(CORES)), **kwargs)
    full = np.empty((nuniq, D), dtype=np.float32)
    for c in range(CORES):
        blk = np.asarray(res.results[c]["out"])  # [128, 2, rp] d-major
        n_c = bounds[c + 1] - bounds[c]
        full[bounds[c]:bounds[c + 1]] = (
            blk.transpose(2, 1, 0).reshape(rp, D)[:n_c].astype(np.float32))
    out = full[inv]
    if _bench is not None:
        kernel.last_results = res
    return out
